# revision 36
# baseline (speedup 1.0000x reference)
"""Multi-head attention Trainium2 Bass kernel (8 NeuronCores), v4.

Problem: nn_MultiHeadAttention (B=2, S=2048, D=1024, H=16, DK=64).

The reference's raw `.view(B, H, S, DK)` reshape makes head h of batch b a
reinterpretation of the contiguous 128-row block x[b, 128h:128h+128, :], so
each (b, h) is an independent attention problem. 32 pairs over 8 cores ->
4 pairs/core, no collectives. Query/key positions are permuted
(s2' = m*128 + r instead of r*16 + m) identically on q and k (softmax is
permutation-invariant along keys) and un-permuted for free by the ctx
layout.

PE cycle floor (fp16, 2.4 GHz, measured 216 ns per 512-wide matmul):
projections 98304c + scores 131072c + ctx 131072c + out-proj 32768c
~= 169 us — the hard wall. Scalar exp wall ~120 us. The kernel is one
software-pipelined instruction stream tuned to keep the in-order PE FIFO
dense from ~12 us:

 - single-queue priority DMA (parallel queues split HBM bandwidth evenly
   and starve the critical prefix): x & Wk-lo kc-pieces interleaved, Wq-lo,
   Wv-lo, Wk-hi, Wv-hi, Wq-hi, Wo; one SBUF tile per piece so dependency
   tracking unblocks consumers per piece.
 - just-in-time lead-in: only k-c0 + q-c0..3 before the first score;
   k-c1..7, v-chips and q-hi stream into pass 0 at mk hooks.
 - 8 attention passes (pair x query-half); ctx matmuls are emitted with a
   TWO-iteration skew after later scores, so the PE never parks on an exp
   semaphore and sem latency amortizes across the 2-slot sw ring.
 - remaining projections, the deferred normalize/broadcast and per-pair
   output projections are spread as <=8-matmul chips at static mk hooks,
   sized to fill the PE slack under the exp cadence.
 - PSUM (one pending accumulation group per 2KB bank — verified hw
   constraint): sw ring 2x[128,1024] + pcA/pcB accumulator banks + a
   2-bank pj ring for projection/out-proj/broadcast psums.
 - softmax: v65 ones column makes pc row 64 the denominator; den rows
   drain to 32-aligned partitions of one tile; ONE exact [97,512] DVE
   reciprocal per pair (split in half for the last pair so its tail
   shortens); broadcast down 64 partitions via PE outer product
   (ones[1,64].T @ rec16[1,512] -> PSUM, 216 ns); DVE multiplies into the
   ctx layout the out-projection consumes directly. DVE reciprocal is
   ~6.5 cyc/elem, so batching all 2048 denominators of a pair into one
   512-free-dim instruction is 16x cheaper than v1's per-quarter calls;
   reciprocal_approx_fast is broken on this hw (measured ~0.4 rel err).
 - projection drains pair the two same-parity (b,h) pairs into single
   strided [64,2,128] copies; kTz/qT2 are single tensors to enable this.
 - scalar engine runs exp almost exclusively; gpsimd (SBUF-only, 32-
   aligned partition windows) does memsets; DVE does everything else.
 - fp16 output, cast + bias on host: halves the tail DMA.

Measured: 240.7 us (v1 baseline) -> ~218.5 us, rel err 1.2e-3.
fp16 matmul operands (fp32 PSUM); bq/bk/bv are zeros by spec; bo on host.
"""

import sys

sys.path.insert(0, "/opt/trn_rl_repo")

import numpy as np

import concourse.bass as bass  # noqa: E402
import concourse.tile as tile  # noqa: E402
from concourse import bacc, mybir  # noqa: E402
from concourse.bass_utils import run_bass_kernel_spmd  # noqa: E402

F16 = mybir.dt.float16
F32 = mybir.dt.float32

B, S, D, H = 2, 2048, 1024, 16
DK = 64
NCORES = 8
NPAIR = 4
R = 128
NM = 16
S2 = NM * R
KC = D // 128
SCALE = 1.0 / np.sqrt(np.float32(DK))


def _build():
    nc = bacc.Bacc("TRN2", target_bir_lowering=False, debug=False,
                   num_devices=NCORES)

    xTa = nc.dram_tensor("xTa", [128, KC * 512], F16, kind="ExternalInput").ap()
    wqa = nc.dram_tensor("wqa", [128, KC * 1024], F16,
                         kind="ExternalInput").ap()
    wka = nc.dram_tensor("wka", [128, KC * 1024], F16,
                         kind="ExternalInput").ap()
    wva = nc.dram_tensor("wva", [128, KC * 1024], F16,
                         kind="ExternalInput").ap()
    woa = nc.dram_tensor("woa", [128, KC * 1024], F16,
                         kind="ExternalInput").ap()
    out = nc.dram_tensor("out", [NPAIR * R, D], F16, kind="ExternalOutput").ap()

    with tile.TileContext(nc) as tc:
        with tc.tile_pool(name="w", bufs=1) as wpool, \
             tc.tile_pool(name="xp", bufs=1) as xpool, \
             tc.tile_pool(name="qk", bufs=1) as qkpool, \
             tc.tile_pool(name="v6", bufs=1) as vpool, \
             tc.tile_pool(name="pt", bufs=1) as ptpool, \
             tc.tile_pool(name="cu", bufs=1) as cupool, \
             tc.tile_pool(name="cx", bufs=1) as cpool, \
             tc.tile_pool(name="ot", bufs=1) as otpool, \
             tc.tile_pool(name="ps", bufs=1, space="PSUM") as pspool:

            # ---------------- SBUF persistent tiles ----------------
            # one tile per DMA piece: DMA-written tiles get tile-granular
            # read dependencies, so a piece-tile unblocks consumers as soon
            # as ITS transfer lands.
            xTp = [xpool.tile([128, 1024], F16, name=f"xT{i}", tag=f"x{i}",
                              bufs=1) for i in range(4)]
            # v5: wq/wk host-packed C-MAJOR (col c*1024 + kc*128 + f) in 8
            # per-chip 0.25MB pieces, so chip c streams as soon as ITS
            # piece lands; wv in 4 kc-half pieces; wo in 2 halves.
            wkp = [wpool.tile([128, 1024], F16, name=f"wk{c}", tag=f"wk{c}",
                              bufs=1) for c in range(8)]
            wqp = [wpool.tile([128, 1024], F16, name=f"wq{c}", tag=f"wq{c}",
                              bufs=1) for c in range(8)]
            wvp = [wpool.tile([128, 2048], F16, name=f"wv{i}", tag=f"wv{i}",
                              bufs=1) for i in range(4)]
            woh = [wpool.tile([128, 4096], F16, name=f"wo{j}", tag=f"wo{j}",
                              bufs=1) for j in range(2)]

            def xop(kc):
                return xTp[kc // 2][:, (kc % 2) * 512:(kc % 2 + 1) * 512]

            def xop_pr(kc, pr):
                o = (kc % 2) * 512 + pr * 128
                return xTp[kc // 2][:, o:o + 128]

            def wk_op(c, kc):
                return wkp[c][:, kc * 128:(kc + 1) * 128]

            def wq_op(c, kc):
                return wqp[c][:, kc * 128:(kc + 1) * 128]

            # v5 layouts for qh-row-tiled scores (scores contract only
            # DK=64, so the two qh score matmuls of one mk run CONCURRENTLY
            # in different PE row-groups — rows 0:64 for qh0, 64:128 for
            # qh1 — halving score time):
            #  kTd: k DUPLICATED in both partition halves; col
            #       pr*S2 + m*128 + r as before.
            #  qTs: query block (h2, qh) at partition half qh, col
            #       pr*1024 + (h2*4 + m%4)*128 + r (no pair-packing).
            qTs = qkpool.tile([128, NPAIR * 1024], F16, name="qTs", tag="q",
                              bufs=1)
            kTd = qkpool.tile([128, NPAIR * S2], F16, name="kTd", tag="kz",
                              bufs=1)
            v65 = [vpool.tile([128, NM * 65], F16, name=f"v65{p}",
                              tag=f"v{p}", bufs=1) for p in range(NPAIR)]
            ctx = [cpool.tile([128, D], F16, name=f"ctx{p}", tag=f"c{p}",
                              bufs=1) for p in range(NPAIR)]
            ones64 = cupool.tile([97, 64], F16, name="ones64", tag="o64",
                                 bufs=1)
            nc.vector.memset(ones64[:], 1.0)

            # ---------------- HAM warmup + exp table prefetch ----------
            # a few dummy matmuls at t~0 keep the PE activity window busy
            # through the DMA lead-in so the clock gate opens (K=8/8,
            # 2.4 GHz) before the real projection chips run; a dummy exp
            # pulls the ~2.7us ACT_TABLE_LOAD off the critical path.
            wrm = cupool.tile([64, 512], F16, name="wrm", tag="wrm", bufs=1)
            nc.vector.memset(wrm[:], 1.0)
            wps = pspool.tile([128, 512], F32, name="wps", tag="pj", bufs=2)
            for _ in range(7):
                nc.tensor.matmul(wps[0:64, :], ones64[0:64, :], wrm[:],
                                 start=True, stop=True)
            dumT = cupool.tile([1, 64], F16, name="dumT", tag="dum", bufs=1)
            nc.scalar.activation(dumT[:], ones64[0:1, :],
                                 mybir.ActivationFunctionType.Exp, scale=1.0)

            # ---------------- input DMA ----------------
            # one queue, strict priority order (parallel queues split the
            # HBM bandwidth evenly and starve the critical prefix).
            # Priority: x + Wk-c0 (first k chip), Wq c0..3 (first act),
            # Wv-g0 (ctx pass 0), Wk c1-2, Wv-g1, Wk c3..7, Wq c4..7
            # (pass 1), Wo (pass 2+).
            nc.sync.dma_start(xTp[0][:], xTa[:, 0:1024])
            nc.sync.dma_start(wkp[0][:], wka[:, 0:1024])
            for i in range(1, 4):
                nc.sync.dma_start(xTp[i][:], xTa[:, i * 1024:(i + 1) * 1024])
            for c in range(4):
                nc.sync.dma_start(wqp[c][:],
                                  wqa[:, c * 1024:(c + 1) * 1024])
            nc.sync.dma_start(wvp[0][:], wva[:, 0:2048])
            nc.sync.dma_start(wvp[1][:], wva[:, 2048:4096])
            nc.sync.dma_start(wkp[1][:], wka[:, 1024:2048])
            nc.sync.dma_start(wkp[2][:], wka[:, 2048:3072])
            nc.sync.dma_start(wvp[2][:], wva[:, 4096:6144])
            nc.sync.dma_start(wvp[3][:], wva[:, 6144:8192])
            for c in range(3, 8):
                nc.sync.dma_start(wkp[c][:],
                                  wka[:, c * 1024:(c + 1) * 1024])
            for c in range(4, 8):
                nc.sync.dma_start(wqp[c][:],
                                  wqa[:, c * 1024:(c + 1) * 1024])
            nc.sync.dma_start(woh[0][:], woa[:, 0:4096])
            nc.sync.dma_start(woh[1][:], woa[:, 4096:8192])

            # v65 ones columns (free softmax denominators).
            for p in range(NPAIR):
                ones_cols = v65[p][:].rearrange("p (m c) -> p m c",
                                                m=NM)[:, :, 64:65]
                nc.gpsimd.memset(ones_cols, 1.0)

            # ---------------- projection helpers ----------------
            chip_ps = {}

            def qk_chip(w_op, c, dst_tiles, tag, kc0, kc1, scalar_ok=False):
                """Emit kc0..kc1 of one qk projection chunk; drains on the
                last chip."""
                key = (w_op, c)
                if kc0 == 0:
                    chip_ps[key] = pspool.tile(
                        [128, 512], F32, name=f"pp{c}", tag=tag,
                        bufs=2 if tag in ("sc", "pj") else 1)
                ps = chip_ps[key]
                for kc in range(kc0, kc1):
                    nc.tensor.matmul(
                        ps[:], w_op(c, kc), xop(kc),
                        start=(kc == 0), stop=(kc == KC - 1))
                if kc1 == KC:
                    qk_drain(ps, c, dst_tiles, scalar_ok)

            def qk_drain(ps, c, dst_t, scalar_ok):
                """v5 drains, one [64,4,128] strided copy covers all 4
                pairs. k: feature block m=2c+mp duplicated into BOTH
                partition halves (each qh score tile needs its own
                stationary copy). q: block m goes only to partition half
                qh(m) = (m//4)%2, col block h2(m)*4 + m%4."""
                tile_t, mode = dst_t
                for mp in range(2):
                    m = 2 * c + mp
                    srcv = ps[mp * 64:mp * 64 + 64, :].rearrange(
                        "p (a r) -> p a r", a=4)
                    if mode == "k":
                        for h in range(2):
                            dstv = tile_t[h * 64:h * 64 + 64, :].rearrange(
                                "p (a g r) -> p a g r", a=4, g=NM)[:, :, m, :]
                            if scalar_ok and h == 1:
                                nc.scalar.copy(dstv, srcv)
                            else:
                                nc.vector.tensor_copy(dstv, srcv)
                    else:
                        qh = (m // 4) % 2
                        dstv = tile_t[qh * 64:qh * 64 + 64, :].rearrange(
                            "p (a u r) -> p a u r", a=4, u=8)[
                            :, :, (m // 8) * 4 + (m % 4), :]
                        if scalar_ok and mp == 1:
                            nc.scalar.copy(dstv, srcv)
                        else:
                            nc.vector.tensor_copy(dstv, srcv)

            def v_chip(pr, g, tag, kc0, kc1):
                key = ("v", pr, g)
                if kc0 == 0:
                    chip_ps[key] = pspool.tile(
                        [128, 512], F32, name=f"pv{pr}{g}", tag=tag,
                        bufs=2 if tag in ("sc", "pj") else 1)
                psv = chip_ps[key]
                for kc in range(kc0, kc1):
                    nc.tensor.matmul(
                        psv[:], xop_pr(kc, pr),
                        wvp[g * 2 + kc // 4][:, (kc % 4) * 512:
                                             (kc % 4 + 1) * 512],
                        start=(kc == 0), stop=(kc == KC - 1))
                if kc1 == KC:
                    dst = v65[pr][:].rearrange(
                        "p (m c) -> p m c", m=NM)[:, g * 8:(g + 1) * 8, 0:64]
                    nc.vector.tensor_copy(
                        dst, psv[:].rearrange("p (m c) -> p m c", m=8))

            def op_chip(pr, jb, c0, c1):
                key = ("o", pr, jb)
                if c0 == 0:
                    chip_ps[key] = pspool.tile([128, 512], F32, name="po",
                                               tag="pj", bufs=2)
                po = chip_ps[key]
                for c in range(c0, c1):
                    nc.tensor.matmul(
                        po[:],
                        ctx[pr][:, c * 128:(c + 1) * 128],
                        woh[jb][:, c * 512:(c + 1) * 512],
                        start=(c == 0), stop=(c == KC - 1))
                if c1 == KC:
                    ot = otpool.tile([128, 512], F16, name="ot", tag="ot",
                                     bufs=4)
                    # drains that land after the final exp go on the (then
                    # idle) scalar engine: the whole last pair, and pair
                    # 2's jb=1 chip whose drain would otherwise queue on
                    # the DVE ahead of the tail reciprocal
                    if pr == NPAIR - 1 or (pr == NPAIR - 2 and jb == 1):
                        nc.scalar.copy(ot[:], po[:])
                    else:
                        nc.vector.tensor_copy(ot[:], po[:])
                    nc.sync.dma_start(
                        out[pr * 128:(pr + 1) * 128,
                            jb * 512:(jb + 1) * 512], ot[:])

            # ---------------- deferred pair-tail work ----------------
            cu = {}
            dens = {}
            state = {}

            def norm_recip(pr):
                """Batched reciprocal of pair pr's denominators + fp16
                cast + qs=3 relay (PE operand base must be 0/32/64)."""
                rec = cupool.tile([97, 512], F32, name="rec", tag="rec",
                                  bufs=2)
                nc.vector.reciprocal(rec[:], dens[pr][:])
                rec16 = cupool.tile([97, 512], F16, name="rec16", tag="r16",
                                    bufs=2)
                nc.vector.tensor_copy(rec16[:], rec[:])
                rec16b = cupool.tile([1, 512], F16, name="rec16b",
                                     tag="r16b", bufs=2)
                nc.vector.tensor_copy(rec16b[:], rec16[96:97, :])
                state[pr] = (rec16, rec16b)

            def norm_recip_part(pr, part):
                """Half-pair reciprocal for the LAST pair (dedicated tags:
                its state lives from pass 4 to the tail while pairs 0..2
                cycle the shared rec ring). fp16-direct output skips the
                cast."""
                if part == 0:
                    rec16 = cupool.tile([97, 512], F16, name="rec16L",
                                        tag="r16L", bufs=1)
                    rec16b = cupool.tile([1, 512], F16, name="rec16bL",
                                         tag="r16bL", bufs=1)
                    state[pr] = (rec16, rec16b)
                    with nc.allow_low_precision(
                            reason="1/den fits fp16; was fp16 via cast"):
                        nc.vector.reciprocal(rec16[0:33, :],
                                             dens[pr][0:33, :])
                else:
                    rec16, rec16b = state[pr]
                    with nc.allow_low_precision(
                            reason="1/den fits fp16; was fp16 via cast"):
                        nc.vector.reciprocal(rec16[64:97, :],
                                             dens[pr][64:97, :])
                    nc.vector.tensor_copy(rec16b[:], rec16[96:97, :])

            def norm_qs(pr, qs, tag="pj"):
                """PE outer-product broadcast of 1/den + DVE multiply into
                the ctx layout. The last pair's final two quarters read
                the pc accumulator PSUM directly (no cu copy)."""
                rec16, rec16b = state[pr]
                pbs = pspool.tile([64, 512], F32, name="pbs", tag=tag,
                                  bufs=2)
                lo = qs * 32 if qs < 3 else 0
                rsrc = rec16[lo:lo + 1, :] if qs < 3 else rec16b[:]
                nc.tensor.matmul(pbs[:], ones64[lo:lo + 1, :], rsrc,
                                 start=True, stop=True)
                src_t = cu[(pr, qs)][:]
                for p2 in range(2):
                    src = src_t.rearrange(
                        "p (a q c) -> p a q c", a=2, q=2)[:, :, p2, :]
                    bb = pbs[:].rearrange(
                        "p (a q c) -> p a q c", a=2, q=2)[:, :, p2, :]
                    dst = ctx[pr][p2 * 64:(p2 + 1) * 64,
                                  qs * 256:(qs + 1) * 256].rearrange(
                        "p (a c) -> p a c", a=2)
                    nc.vector.tensor_mul(dst, src, bb)

            def outproj(pr, jb):
                po = pspool.tile([128, 512], F32, name="po", tag="pj",
                                 bufs=2)
                for c in range(KC):
                    nc.tensor.matmul(
                        po[:],
                        ctx[pr][:, c * 128:(c + 1) * 128],
                        wo[:, c * 1024 + jb * 512:c * 1024 + (jb + 1) * 512],
                        start=(c == 0), stop=(c == KC - 1))
                ot = otpool.tile([128, 512], F16, name="ot", tag="ot",
                                 bufs=4)
                nc.vector.tensor_copy(ot[:], po[:])
                nc.sync.dma_start(
                    out[pr * 128:(pr + 1) * 128,
                        jb * 512:(jb + 1) * 512], ot[:])

            # ---------------- phase 1 lead-in ----------------
            # k c0..3 in kc-halves (streams behind the split wkA DMA,
            # 4 open psums), then q c0..3, then v(pair0) lo half.
            KDST = (kTd, "k")     # pair p at columns p*S2, dup halves
            QDST = (qTs, "q")     # pair p at columns p*1024, qh halves
            qk_chip(wk_op, 0, KDST, "sc", 0, 4, scalar_ok=True)
            qk_chip(wk_op, 0, KDST, "sc", 4, 8, scalar_ok=True)
            for c, tg in zip(range(4), ("pcA", "pcB", "sc", "pj")):
                qk_chip(wq_op, c, QDST, tg, 0, 4, scalar_ok=True)
                qk_chip(wq_op, c, QDST, tg, 4, 8, scalar_ok=True)

            # ------------- static fill schedule (pass, mk) -> thunks -----
            # Every chip is <= 8 matmuls; placed so its inputs (DMA pieces,
            # the DVE reciprocal chain, normalized ctx) are ready before
            # the PE FIFO reaches it.
            SCHED = {}

            def at(pi, mk, fn):
                SCHED.setdefault((pi, mk), []).append(fn)

            # pass 0: k c1..7 at 4-MM hooks paced to the score deadline
            # (chip cJ feeds score mk=2J).
            for j in range(1, 8):
                at(0, 2 * j - 2,
                   lambda c=j: qk_chip(wk_op, c, KDST, "pj", 0, 4))
                at(0, 2 * j - 1,
                   lambda c=j: qk_chip(wk_op, c, KDST, "pj", 4, 8))
            # v(p) streams inside ITS h2=0 pass (ctx(mk0) pops at mk2,
            # ctx(mk8) at mk10 -- halves at hooks 0/1 and 8/9 make it).
            for p in range(NPAIR):
                at(p, 0, lambda p=p: v_chip(p, 0, "pj", 0, 4))
                at(p, 1, lambda p=p: v_chip(p, 0, "pj", 4, 8))
                at(p, 8, lambda p=p: v_chip(p, 1, "pj", 0, 4))
                at(p, 9, lambda p=p: v_chip(p, 1, "pj", 4, 8))
            # q c4..7 (the h2=1 query blocks) spread over passes 1..3
            # (deadline: pass 4 mk0), 4-MM pieces.
            for c, (pA, hA, pB, hB) in {4: (1, 12, 1, 13),
                                        5: (2, 12, 2, 13),
                                        6: (3, 2, 3, 3),
                                        7: (3, 10, 3, 11)}.items():
                at(pA, hA, lambda c=c: qk_chip(wq_op, c, QDST, "pj", 0, 4))
                at(pB, hB, lambda c=c: qk_chip(wq_op, c, QDST, "pj", 4, 8))
            # pair pr (0..2) drains at the flush of pass 4+pr; the recip
            # now precedes the cu copies on the DVE, so rec16 is ready
            # ~mk4 of pass 5+pr. Normalize at mks 5-8, outproj spread
            # thin (2-3 MMs/hook) over mks 9-15.
            for pr in range(NPAIR - 1):
                pi = 5 + pr
                for qs in range(4):
                    at(pi, 6 + qs, lambda pr=pr, qs=qs: norm_qs(pr, qs))
                at(pi, 10, lambda pr=pr: op_chip(pr, 0, 0, 3))
                at(pi, 11, lambda pr=pr: op_chip(pr, 0, 3, 6))
                at(pi, 12, lambda pr=pr: op_chip(pr, 0, 6, 8))
                at(pi, 13, lambda pr=pr: op_chip(pr, 1, 0, 3))
                at(pi, 14, lambda pr=pr: op_chip(pr, 1, 3, 6))
                at(pi, 15, lambda pr=pr: op_chip(pr, 1, 6, 8))
            # pair 3 h2=0 drains at the pass-4 flush: normalize qs0/1
            # there once the part-0 recip has cleared the DVE; its full
            # outproj runs at the tail (c0..3 hide under the final recip).
            at(4, 11, lambda: norm_qs(3, 0))
            at(4, 12, lambda: norm_qs(3, 1))

            # ------------- attention: 8 skewed passes --------------------
            def drain_pass(pc, pr, h2):
                """Drains of a finished pass: den rows (scalar), ctx rows
                (DVE, ahead of the reciprocal so the next pass's ctx isn't
                queued behind it), then the reciprocal chain."""
                last = (pr == NPAIR - 1)
                if h2 == 0:
                    # h2-grouped order keeps all 4 pairs' dens/cu-lo alive
                    # from their h0 pass until pass 5+pr: ring of 4.
                    dens[pr] = cupool.tile([97, 512], F32, name="den",
                                           tag="den", bufs=4)
                # At the very last drain (no next pass needs the pc banks)
                # the reciprocal jumps the DVE queue ahead of the cu
                # copies; everywhere else the cu copies go first so the pc
                # banks free up for the next pass's ctx accumulation.
                tail_drain = last and h2 == 1
                for qh in range(2):
                    qs = h2 * 2 + qh
                    nc.vector.tensor_copy(dens[pr][qs * 32:qs * 32 + 1, :],
                                          pc[qh][64:65, :])
                if tail_drain:
                    norm_recip_part(pr, h2)
                for qh in range(2):
                    qs = h2 * 2 + qh
                    c_t = cupool.tile([64, 512], F32, name=f"cu{pr}{qs}",
                                      tag=f"cu{qs}",
                                      bufs=4 if h2 == 0 else 2)
                    nc.vector.tensor_copy(c_t[:], pc[qh][0:64, :])
                    cu[(pr, qs)] = c_t
                if not tail_drain:
                    if last:
                        norm_recip_part(pr, h2)
                    elif h2 == 1:
                        norm_recip(pr)

            # two-iteration ctx skew: emit ctx(i-2) after scores(i), so
            # the in-order PE FIFO never parks on an exp semaphore and the
            # sem latency amortizes across the 2-slot sw ring.
            pend = []          # [(thunk, pass_info_if_last | None), ...]
            # h2-grouped pass order: all h2=0 passes first. This spreads
            # the front-loaded chip demand (k chips + v0 in pass 0, one v
            # per pass after, q-hi in pass 3) so the scalar exp stream
            # stays dense; pairs normalize/out-project in passes 4..7.
            PASS_SEQ = [(p, 0) for p in range(NPAIR)] + \
                       [(p, 1) for p in range(NPAIR)]
            for pi, (pr, h2) in enumerate(PASS_SEQ):
                pc = [pspool.tile([65, 512], F32, name=f"pc{qh}",
                                  tag=("pcA", "pcB")[qh], bufs=1)
                      for qh in range(2)]
                for mk in range(NM):
                    sw = pspool.tile([128, 1024], F32, name="sw",
                                     tag="sc", bufs=2)
                    # the two qh matmuls contract only 64 partitions each
                    # (row groups 0-1 for qh0, 2-3 for qh1 via the auto
                    # tile_position) and stream CONCURRENTLY in the array.
                    for qh in range(2):
                        nc.tensor.matmul(
                            sw[:, qh * 512:(qh + 1) * 512],
                            kTd[qh * 64:qh * 64 + 64,
                                pr * S2 + mk * 128:pr * S2 + (mk + 1) * 128],
                            qTs[qh * 64:qh * 64 + 64,
                                pr * 1024 + h2 * 512:
                                pr * 1024 + (h2 + 1) * 512],
                            start=True, stop=True)
                    pT = ptpool.tile([128, 1024], F16, name="pT",
                                     tag="pt", bufs=6)
                    nc.scalar.activation(
                        pT[:], sw[:], mybir.ActivationFunctionType.Exp,
                        scale=float(SCALE))
                    if len(pend) == 2:
                        fn, fin = pend.pop(0)
                        fn()
                        if fin is not None:
                            drain_pass(*fin)

                    def make_ctx(pc=pc, pr=pr, mk=mk, pT=pT):
                        for qh in range(2):
                            nc.tensor.matmul(
                                pc[qh][:],
                                v65[pr][:, mk * 65:(mk + 1) * 65],
                                pT[:, qh * 512:(qh + 1) * 512],
                                start=(mk == 0), stop=(mk == NM - 1))
                    pend.append((make_ctx,
                                 (pc, pr, h2) if mk == NM - 1 else None))
                    for fn in SCHED.get((pi, mk), []):
                        fn()

            # tail: flush skewed ctx + last drains (kicks recip part 1).
            # The last pair's outproj low c-chunks depend only on qs0/1
            # (normalized in pass 4), so they execute during the DVE
            # reciprocal window and keep the PE clock warm.
            for fn, fin in pend:
                fn()
                if fin is not None:
                    drain_pass(*fin)
            op_chip(3, 0, 0, 4)
            op_chip(3, 1, 0, 4)
            # warm-keepers: bridge the rest of the DVE recip window so the
            # HAM clock gate stays open for the final norm/outproj matmuls
            # (sc-ring bank -- the pj ring is parked by the op chips)
            wkt = pspool.tile([128, 512], F32, name="wkt", tag="sc", bufs=2)
            for _ in range(8):
                nc.tensor.matmul(wkt[0:64, :], ones64[0:64, :], wrm[:],
                                 start=True, stop=True)
            norm_qs(3, 2, tag="sc")
            for _ in range(4):
                nc.tensor.matmul(wkt[0:64, :], ones64[0:64, :], wrm[:],
                                 start=True, stop=True)
            norm_qs(3, 3, tag="sc")
            op_chip(3, 0, 4, 8)
            op_chip(3, 1, 4, 8)

    nc.compile()
    return nc


_CACHE = {}


def _get_nc():
    if "nc" not in _CACHE:
        _CACHE["nc"] = _build()
    return _CACHE["nc"]


def _kc_block(a, cols):
    """[1024, cols] -> [128, 8*cols] with kc blocks along columns."""
    return np.ascontiguousarray(
        a.reshape(KC, 128, cols).transpose(1, 0, 2).reshape(128, KC * cols))


def _half_block(a):
    """[1024, 1024] W.T -> [128, 8192] half-major: col h*4096 + kc*512 + f
    holds W.T[kc*128+p, h*512+f]."""
    return np.ascontiguousarray(
        a.reshape(KC, 128, 2, 512).transpose(1, 2, 0, 3).reshape(128, 8192))


def _c_block(a):
    """[1024, 1024] W.T -> [128, 8192] c-major: col c*1024 + kc*128 + f
    holds W.T[kc*128+p, c*128+f] (per-chip 0.25MB DMA pieces)."""
    return np.ascontiguousarray(
        a.reshape(KC, 128, KC, 128).transpose(1, 2, 0, 3).reshape(128, 8192))


def _prep_inputs(x, Wq, Wk, Wv, Wo):
    x = np.asarray(x, dtype=np.float32)
    wqa = _c_block(np.ascontiguousarray(Wq.T, dtype=np.float16))
    wka = _c_block(np.ascontiguousarray(Wk.T, dtype=np.float16))
    wva = _half_block(np.ascontiguousarray(Wv.T, dtype=np.float16))
    woa = _half_block(np.ascontiguousarray(Wo.T, dtype=np.float16))

    in_maps = []
    for core in range(NCORES):
        b, hg = core // 4, core % 4
        rows = x[b, hg * 512:(hg + 1) * 512, :]
        xTa = _kc_block(np.ascontiguousarray(rows.T.astype(np.float16)), 512)
        in_maps.append({
            "xTa": xTa, "wqa": wqa, "wka": wka, "wva": wva, "woa": woa,
        })
    return in_maps


def _run(in_maps, trace=False):
    nc = _get_nc()
    return run_bass_kernel_spmd(nc, in_maps, core_ids=list(range(NCORES)),
                                trace=trace)


def kernel(x, Wq, bq, Wk, bk, Wv, bv, Wo, bo, _trace=False):
    x = np.asarray(x, dtype=np.float32)
    in_maps = _prep_inputs(x, np.asarray(Wq), np.asarray(Wk),
                           np.asarray(Wv), np.asarray(Wo))
    res = _run(in_maps, trace=_trace)
    out = np.empty((B, S, D), dtype=np.float32)
    for core in range(NCORES):
        b, hg = core // 4, core % 4
        out[b, hg * 512:(hg + 1) * 512, :] = res.results[core]["out"]
    out += np.asarray(bo, dtype=np.float32)[None, None, :]
    kernel.last_result = res
    return out

